# revision 24
# baseline (speedup 1.0000x reference)
"""Trainium2 Bass kernel for nn_AgeConditionedGraphPriorLoss.

Strategy (final)
----------------
logits (2, 32, 96, 96, 96) fp32 is the only large tensor (~216 MiB); the
problem is memory-bound.  Shard over (batch B=2) x (four Y-slabs of 24)
across 8 NeuronCores; each core keeps the full X range so the flip/swap
symmetry term is shard-local.

Host prep: shards are pre-transposed to [NITER, 128, CHUNK*C*VT], cast to
fp8 e4m3 (softmax normalizes the same quantized values, so row sums are
still exactly 1), and the descending-x half has its channel halves
pre-swapped (the LR pair permutation), so every device access pattern is
contiguous.

Per core, NITER=6 iterations process an (x ascending, x descending)
chunk pair of CHUNK=8 slabs each:
  * ACT:  e = exp(logit) -> bf16, channel-major [P, C, j, (x,vt)]
  * DVE:  s = sum_c e as a 5-level binary tree of contiguous halving
          bf16 adds (2x packing); t = 1/s via the custom
          reciprocal_approx_fast op, emitted straight to bf16;
          p = e * t in one mul with t broadcast over channels
  * DVE:  symmetry via sum|a-b| = 2*sum max(a,b) - sum a - sum b; with
          softmax rows summing to 1, sum a + sum b is the exact voxel
          count, so one bf16 max per iteration is all the element work
  * PE:   sum max reduced by ones-vector matmuls accumulating into a
          [1,512] PSUM row; gram matmuls packed 4 vtiles wide
          ([128,128]^T[128,128], 432 total) into two PSUM banks (the
          descending chunk's gram is channel-swapped; host unpermutes)
Volumes are gram row sums (softmax rows sum to 1).  Iteration 0 is
split into half-DMAs / half-exps / quarter-L1s to shorten the pipeline
ramp, and the last iteration reorders mul/gram so the Tensor queue
drains early.  The tiny O(C^2) final loss math runs on host in numpy.
"""

import os
import sys

import numpy as np
from contextlib import ExitStack

# kernel.py is graded from a bare directory: make the concourse/bass stack
# importable regardless of cwd
for _p in ("/opt/trn_rl_repo", "/root/.axon_site/_ro/trn_rl_repo"):
    if os.path.isdir(_p) and _p not in sys.path:
        sys.path.append(_p)

# ---- problem constants (hardcoded per harness contract) ----
B = 2
C = 32
X = 96
Y = 96
Z = 96
N_CORES = 8
YQ = Y // 4          # y-slab per core
P = 128              # SBUF partitions

LAMBDA_VOLUME = 0.2
LAMBDA_WEIGHTED_ADJ = 0.15
LAMBDA_SYM = 0.05
AGE_MAX = 100.0
EPS_ROW = 1e-8
EPS_STD = 1e-6

CHUNK = 8            # x-slabs per chunk half
U = 4                # vtiles packed per gram matmul


def build_nc(Cc=C, XS=X, YQc=YQ, Zc=Z):
    """Build the per-core Bass program (SPMD: same program on all cores).

    Inputs : "lg_a" [NITER, 128, CHUNK*Cc*VT] bf16  (ascending x chunks)
             "lg_b" [NITER, 128, CHUNK*Cc*VT] bf16  (descending x chunks)
    Outputs: "a_out"   [128, 128] fp32  (packed gram blocks, diag extract)
             "sym_out" [128, NITER] fp32 (per-partition sum-max partials)
    """
    import concourse.bass as bass
    import concourse.bacc as bacc
    import concourse.tile as tile
    from concourse import mybir
    from concourse.alu_op_type import AluOpType
    from concourse.dve_ops import (
        RECIPROCAL_APPROX_FAST,
        RECIP_APPROX_FAST_CONSTS,
    )

    f32 = mybir.dt.float32
    bf16 = mybir.dt.bfloat16
    f8 = mybir.dt.float8e4

    NV = YQc * Zc                 # voxels per x-slab
    assert NV % P == 0
    VT = NV // P                  # 128-voxel tiles per x-slab
    assert XS % (2 * CHUNK) == 0
    NITER = XS // (2 * CHUNK)
    CH = Cc // 2
    XV = CHUNK * VT               # voxel-groups per chunk (= 72)
    G = XV // U                   # gram groups per chunk (= 18)
    CSLAB = CHUNK * Cc * VT       # elements per chunk per partition

    nc = bacc.Bacc("TRN2", target_bir_lowering=False)
    lg_a = nc.dram_tensor("lg_a", [NITER, P, CSLAB], f8, kind="ExternalInput")
    lg_b = nc.dram_tensor("lg_b", [NITER, P, CSLAB], f8, kind="ExternalInput")
    a_out = nc.dram_tensor("a_out", [2, P, P], f32, kind="ExternalOutput")
    sym_out = nc.dram_tensor("sym_out", [1, 512], f32, kind="ExternalOutput")

    lg_dma_ring = []

    def load_chunk(pool, src, it, split):
        # one chunk half: [P, CHUNK, Cc, VT]; fully contiguous per partition.
        # split=True issues two half-DMAs so the first exp can start earlier
        # (pipeline ramp for iteration 0).
        t = pool.tile([P, CHUNK, Cc, VT], f8, tag="lg")
        HS = CSLAB // 2
        if split:
            for h in range(2):
                s = bass.AP(
                    tensor=src,
                    offset=it * P * CSLAB + h * HS,
                    ap=[[CSLAB, P], [1, HS]],
                )
                d = nc.sync.dma_start(
                    out=t[:, h * (CHUNK // 2) : (h + 1) * (CHUNK // 2)], in_=s
                )
                lg_dma_ring.append(d)
        else:
            s = bass.AP(
                tensor=src,
                offset=it * P * CSLAB,
                ap=[[CSLAB, P], [1, CSLAB]],
            )
            d = nc.sync.dma_start(out=t[:], in_=s)
            lg_dma_ring.append(d)
        return t

    with tile.TileContext(nc) as tc, ExitStack() as ctx:
        lg_pool = ctx.enter_context(tc.tile_pool(name="lg", bufs=4))
        e_pool = ctx.enter_context(tc.tile_pool(name="e", bufs=3))
        p_pool = ctx.enter_context(tc.tile_pool(name="p", bufs=3))
        st_pool = ctx.enter_context(tc.tile_pool(name="st", bufs=1))
        st34_pool = ctx.enter_context(tc.tile_pool(name="st34", bufs=2))
        sm_pool = ctx.enter_context(tc.tile_pool(name="sm", bufs=2))
        m_pool = ctx.enter_context(tc.tile_pool(name="m", bufs=2))
        one_pool = ctx.enter_context(tc.tile_pool(name="one", bufs=1))
        ps_pool = ctx.enter_context(tc.tile_pool(name="ps", bufs=1, space="PSUM"))

        a_psum = ps_pool.tile([P, P], f32)
        b_psum = ps_pool.tile([P, P], f32)
        sym_psum = ps_pool.tile([P, 512], f32)
        a_sb = one_pool.tile([P, P], f32)
        b_sb = one_pool.tile([P, P], f32)
        sym_sb = one_pool.tile([P, 512], f32)
        ones_t = one_pool.tile([P, 1], bf16)
        nc.vector.memset(ones_t[:], 1.0)

        n_mm = NITER * 2 * G
        state = {"mm": 0, "sym_mm": 0}
        F2 = 2 * XV               # tree batch size (both chunks, = 288)
        m_len = 2 * G * CH * U
        sym_offs = list(range(0, m_len, 512))
        n_sym_mm = len(sym_offs)

        def head(it):
            """DMA + exp + tree levels 1-2 (DVE) and 3-4 (GpSimd)."""
            split = it == 0
            lg_ta = load_chunk(lg_pool, lg_a, it, split)
            lg_tb = load_chunk(lg_pool, lg_b, it, split)

            # exp, channel-major out: e [P, Cc, j, (x, vt)]; iteration 0
            # splits each exp by x-half to shorten the pipeline ramp.
            e_t = e_pool.tile([P, Cc, 2, XV], bf16, tag="e")
            for j, lg_t in enumerate((lg_ta, lg_tb)):
                ev = e_t[:, :, j, :].rearrange("p c (x v) -> p c x v", v=VT)
                lv = lg_t[:].transpose([0, 2, 1, 3])
                if split:
                    qx = CHUNK // 4
                    for h in range(4):
                        nc.scalar.activation(
                            out=ev[:, :, h * qx : (h + 1) * qx, :],
                            in_=lv[:, :, h * qx : (h + 1) * qx, :],
                            func=mybir.ActivationFunctionType.Exp,
                        )
                else:
                    nc.scalar.activation(
                        out=ev, in_=lv,
                        func=mybir.ActivationFunctionType.Exp,
                    )

            # channel-sum tree: 32 -> 16 (DVE; j-split on iteration 0 so the
            # first add starts before the second chunk's exp lands)
            st1 = st_pool.tile([P, CH, 2, XV], bf16, tag="st1")
            if split:
                hu = XV // 2
                for j in range(2):
                    for h in range(2):
                        nc.vector.tensor_add(
                            st1[:, :, j, h * hu : (h + 1) * hu],
                            e_t[:, 0:CH, j, h * hu : (h + 1) * hu],
                            e_t[:, CH:Cc, j, h * hu : (h + 1) * hu],
                        )
            else:
                ef = e_t[:].rearrange("p c j u -> p (c j u)")
                n = CH * F2
                nc.vector.tensor_add(
                    st1[:].rearrange("p c j u -> p (c j u)"),
                    ef[:, 0:n],
                    ef[:, n : 2 * n],
                )
            # 16 -> 8 (DVE)
            st1f = st1[:].rearrange("p c j u -> p (c j u)")
            n = CH * F2 // 2
            st2 = st_pool.tile([P, n], bf16, tag="st2")
            nc.vector.tensor_add(st2[:], st1f[:, 0:n], st1f[:, n : 2 * n])
            # 8 -> 4 -> 2 (DVE)
            n //= 2
            st3 = st34_pool.tile([P, n], bf16, tag="st3")
            nc.vector.tensor_add(st3[:], st2[:, 0:n], st2[:, n : 2 * n])
            n //= 2
            st4 = st34_pool.tile([P, n], bf16, tag="st4")
            nc.vector.tensor_add(st4[:], st3[:, 0:n], st3[:, n : 2 * n])
            return e_t, st4

        def tail(it, e_t, st4):
            """Tree level 5 + reciprocal + normalize + symmetry + matmuls."""
            s_f = sm_pool.tile([P, F2], f32, tag="s")
            nc.vector.tensor_add(s_f[:], st4[:, 0:F2], st4[:, F2 : 2 * F2])

            # t = 1/s (fast NR approx, ~51 ULP), bf16 out
            t_b = sm_pool.tile([P, F2], bf16, tag="tb")
            _rc = RECIP_APPROX_FAST_CONSTS
            nc.vector._custom_dve(
                RECIPROCAL_APPROX_FAST,
                out=t_b[:],
                in0=s_f[:],
                s0=_rc["s0"],
                s1=_rc["s1"],
                imm2=_rc["imm2"],
            )

            # p = e * t; out layout [P, j, G, c, U].  On the last iteration
            # the mul is split by chunk so the j=0 gram matmuls start
            # draining the Tensor queue while the DVE finishes j=1 + max.
            p_t = p_pool.tile([P, 2, G, Cc, U], bf16, tag="p")
            p_w = p_t[:].rearrange("p j g c u -> p (j g) c u")
            e_r = e_t[:].rearrange("p c j (g u) -> p (j g) c u", u=U)
            t_r = (
                t_b[:]
                .rearrange("p (j g u) -> p (j g) u", j=2, u=U)
                .unsqueeze(2)
                .broadcast_to([P, 2 * G, Cc, U])
            )

            def emit_gram(j):
                dst = a_psum if j == 0 else b_psum
                for g in range(G):
                    pv = p_t[:, j, g].rearrange("p c u -> p (c u)")
                    nc.tensor.matmul(
                        dst[:],
                        pv,
                        pv,
                        start=(it == 0 and g == 0),
                        stop=(it == NITER - 1 and g == G - 1),
                    )
                    state["mm"] += 1

            last = it == NITER - 1
            if last:
                nc.vector.tensor_mul(
                    p_w[:, 0:G], e_r[:, 0:G], t_r[:, 0:G]
                )
                emit_gram(0)
                nc.vector.tensor_mul(
                    p_w[:, G : 2 * G], e_r[:, G : 2 * G], t_r[:, G : 2 * G]
                )
                emit_gram(1)
            else:
                nc.vector.tensor_mul(p_w, e_r, t_r)

            # symmetry: sum|pa - pb_sigma| = 2*sum max - const; the b chunk
            # is channel-half-swapped on host so this is one plain max
            m_t = m_pool.tile([P, 2, G, CH, U], bf16, tag="m")
            nc.vector.tensor_tensor(
                out=m_t[:].rearrange("p a g c u -> p (a g) c u"),
                in0=p_t[:, 0],
                in1=p_t[:, 1],
                op=AluOpType.max,
            )
            m_flat = m_t[:].rearrange("p a g c u -> p (a g c u)")
            for off in sym_offs:
                w = min(512, m_len - off)
                nc.tensor.matmul(
                    sym_psum[0:1, 0:w],
                    ones_t[:, 0:1],
                    m_flat[:, off : off + w],
                    start=(state["sym_mm"] == 0),
                    stop=(state["sym_mm"] == NITER * n_sym_mm - 1),
                )
                state["sym_mm"] += 1

            if not last:
                emit_gram(0)
                emit_gram(1)

        for it in range(NITER):
            e_t, st4 = head(it)
            tail(it, e_t, st4)

        assert state["mm"] == n_mm
        nc.vector.tensor_copy(out=a_sb[:], in_=a_psum[:])
        nc.sync.dma_start(out=a_out[0], in_=a_sb[:])
        nc.vector.tensor_copy(out=b_sb[:], in_=b_psum[:])
        nc.sync.dma_start(out=a_out[1], in_=b_sb[:])
        nc.vector.tensor_copy(out=sym_sb[0:1, :], in_=sym_psum[0:1, :])
        nc.sync.dma_start(out=sym_out[:], in_=sym_sb[0:1, :])

    # The HWDGE pseudo-DMA has a single sync-wait slot, but a recycled load
    # buffer carries both a WAR wait and a WAW wait.  All SP-issued HWDGE
    # DMAs share one physical FIFO ring, so same-ring WAW ordering is
    # guaranteed by hardware; drop the redundant DMAHW wait.
    for d in lg_dma_ring:
        si = d.ins.sync_info
        if si is None or si.on_wait is None:
            continue
        ws = list(si.on_wait)
        if len(ws) > 1:
            keep = [w for w in ws if not (w.ant_name or "").startswith("DMAHW")]
            if keep and len(keep) < len(ws):
                si.on_wait = keep

    nc.compile()
    return nc


def _finish_loss(A_b, vol_b, sym_total, age, w_young, w_old,
                 vol_means_young, vol_means_old, vol_stds_young, vol_stds_old,
                 prior_adj):
    """Host-side tiny final math (numpy, float64 internally)."""
    alpha = np.clip(age.astype(np.float64) / AGE_MAX, 0.0, 1.0)  # (B,1)

    eye = np.eye(C)
    A = A_b * (1.0 - eye)[None]                                   # zero diag
    W = (1.0 - alpha)[:, :, None] * w_young[None] + alpha[:, :, None] * w_old[None]
    Aw = (A * W).mean(axis=0)
    Aw = Aw / np.clip(Aw.sum(axis=1, keepdims=True), EPS_ROW, None)
    prior = prior_adj * (1.0 - eye)
    prior = prior / np.clip(prior.sum(axis=1, keepdims=True), EPS_ROW, None)
    loss_adj = np.mean(np.abs(Aw - prior))

    means = (1.0 - alpha) * vol_means_young[None] + alpha * vol_means_old[None]
    stds = (1.0 - alpha) * vol_stds_young[None] + alpha * vol_stds_old[None]
    r = (vol_b - means) / (stds + EPS_STD)
    ar = np.abs(r)
    loss_vol = np.mean(np.where(ar < 1.0, 0.5 * r * r, ar - 0.5))

    loss_sym = sym_total / float(B * C * X * Y * Z)

    total = (LAMBDA_WEIGHTED_ADJ * loss_adj
             + LAMBDA_VOLUME * loss_vol
             + LAMBDA_SYM * loss_sym)
    return np.float32(total)


def _shard_for_core(logits, b, q, Cc=C, XS=X, YQc=YQ, Zc=Z):
    """Slice one core's shard into (lg_a, lg_b): ascending / descending
    chunk tensors [NITER, 128, CHUNK*C*VT] bf16 with voxel v = y*Zc + z
    mapped to (vt, part) = (v // 128, v % 128)."""
    NV = YQc * Zc
    VT = NV // P
    NITER = XS // (2 * CHUNK)
    sh = logits[b, :, :, q * YQc : (q + 1) * YQc, :]      # [C, XS, YQ, Z]
    sh = sh.reshape(Cc, XS, VT, P)                        # v -> (vt, part)
    sh = sh.transpose(1, 3, 0, 2)                         # [XS, part, C, VT]
    import ml_dtypes
    sh = np.asarray(sh, dtype=np.float32).astype(ml_dtypes.float8_e4m3)
    asc = sh[: XS // 2].reshape(NITER, CHUNK, P, Cc, VT)
    # descending shard: swap channel halves (the LR pair permutation) so the
    # on-device symmetry max needs no swizzled access pattern
    perm = np.concatenate([np.arange(Cc // 2, Cc), np.arange(0, Cc // 2)])
    dsc = sh[XS // 2 :][::-1][:, :, perm]
    dsc = dsc.reshape(NITER, CHUNK, P, Cc, VT)
    # [NITER, P, CHUNK, C, VT] flattened per partition
    lg_a = np.ascontiguousarray(asc.transpose(0, 2, 1, 3, 4)).reshape(
        NITER, P, CHUNK * Cc * VT
    )
    lg_b = np.ascontiguousarray(dsc.transpose(0, 2, 1, 3, 4)).reshape(
        NITER, P, CHUNK * Cc * VT
    )
    return lg_a, lg_b


_CACHE = {}


def kernel(logits, age, w_young, w_old, vol_means_young, vol_means_old,
           vol_stds_young, vol_stds_old, prior_adj, perm):
    from concourse.bass_utils import run_bass_kernel_spmd

    logits = np.asarray(logits, dtype=np.float32)

    if "nc" not in _CACHE:
        _CACHE["nc"] = build_nc()
    nc = _CACHE["nc"]

    in_maps = []
    for core in range(N_CORES):
        b = core // 4
        q = core % 4
        la, lb = _shard_for_core(logits, b, q)
        in_maps.append({"lg_a": la, "lg_b": lb})

    res = run_bass_kernel_spmd(nc, in_maps, core_ids=list(range(N_CORES)))
    _CACHE["last_results"] = res

    NVOX_CORE = X * YQ * Z
    A_b = np.zeros((B, C, C), dtype=np.float64)
    sym_total = 0.0
    for core in range(N_CORES):
        b = core // 4
        a_full = res.results[core]["a_out"].astype(np.float64)
        # a_full[j, 4*c1+u1, 4*c2+u2]: diagonal u1==u2 blocks are the gram;
        # the j=1 (descending) gram is channel-half-swapped -> unpermute
        perm = np.concatenate([np.arange(C // 2, C), np.arange(0, C // 2)])
        Aa = np.einsum("cudu->cd", a_full[0].reshape(C, U, C, U))
        Ab = np.einsum("cudu->cd", a_full[1].reshape(C, U, C, U))
        A_b[b] += Aa + Ab[np.ix_(perm, perm)]
        sum_max = float(res.results[core]["sym_out"].astype(np.float64).sum())
        sym_core = 2.0 * sum_max - NVOX_CORE
        sym_total += 2.0 * sym_core
    vol_b = A_b.sum(axis=2)  # softmax rows sum to 1 -> row sums give volumes

    return _finish_loss(
        A_b, vol_b, sym_total,
        np.asarray(age), np.asarray(w_young), np.asarray(w_old),
        np.asarray(vol_means_young), np.asarray(vol_means_old),
        np.asarray(vol_stds_young), np.asarray(vol_stds_old),
        np.asarray(prior_adj),
    )


# revision 25
# speedup vs baseline: 1.0028x; 1.0028x over previous
"""Trainium2 Bass kernel for nn_AgeConditionedGraphPriorLoss.

Strategy (final)
----------------
logits (2, 32, 96, 96, 96) fp32 is the only large tensor (~216 MiB); the
problem is memory-bound.  Shard over (batch B=2) x (four Y-slabs of 24)
across 8 NeuronCores; each core keeps the full X range so the flip/swap
symmetry term is shard-local.

Host prep: shards are pre-transposed to [NITER, 128, CHUNK*C*VT], cast to
fp8 e4m3 (softmax normalizes the same quantized values, so row sums are
still exactly 1), and the descending-x half has its channel halves
pre-swapped (the LR pair permutation), so every device access pattern is
contiguous.

Per core, NITER=6 iterations process an (x ascending, x descending)
chunk pair of CHUNK=8 slabs each:
  * ACT:  e = exp(logit) -> bf16, channel-major [P, C, j, (x,vt)]
  * DVE:  s = sum_c e as a 5-level binary tree of contiguous halving
          bf16 adds (2x packing); t = 1/s via the custom
          reciprocal_approx_fast op, emitted straight to bf16;
          p = e * t in one mul with t broadcast over channels
  * DVE:  symmetry via sum|a-b| = 2*sum max(a,b) - sum a - sum b; with
          softmax rows summing to 1, sum a + sum b is the exact voxel
          count, so one bf16 max per iteration is all the element work
  * PE:   sum max reduced by ones-vector matmuls accumulating into a
          [1,512] PSUM row; gram matmuls packed 4 vtiles wide
          ([128,128]^T[128,128], 432 total) into two PSUM banks (the
          descending chunk's gram is channel-swapped; host unpermutes)
Volumes are gram row sums (softmax rows sum to 1).  Iteration 0 is
split into half-DMAs / half-exps / quarter-L1s to shorten the pipeline
ramp, and the last iteration reorders mul/gram so the Tensor queue
drains early.  The tiny O(C^2) final loss math runs on host in numpy.
"""

import os
import sys

import numpy as np
from contextlib import ExitStack

# kernel.py is graded from a bare directory: make the concourse/bass stack
# importable regardless of cwd
for _p in ("/opt/trn_rl_repo", "/root/.axon_site/_ro/trn_rl_repo"):
    if os.path.isdir(_p) and _p not in sys.path:
        sys.path.append(_p)

# ---- problem constants (hardcoded per harness contract) ----
B = 2
C = 32
X = 96
Y = 96
Z = 96
N_CORES = 8
YQ = Y // 4          # y-slab per core
P = 128              # SBUF partitions

LAMBDA_VOLUME = 0.2
LAMBDA_WEIGHTED_ADJ = 0.15
LAMBDA_SYM = 0.05
AGE_MAX = 100.0
EPS_ROW = 1e-8
EPS_STD = 1e-6

CHUNK = 8            # x-slabs per chunk half
U = 4                # vtiles packed per gram matmul


def build_nc(Cc=C, XS=X, YQc=YQ, Zc=Z):
    """Build the per-core Bass program (SPMD: same program on all cores).

    Inputs : "lg_a" [NITER, 128, CHUNK*Cc*VT] bf16  (ascending x chunks)
             "lg_b" [NITER, 128, CHUNK*Cc*VT] bf16  (descending x chunks)
    Outputs: "a_out"   [128, 128] fp32  (packed gram blocks, diag extract)
             "sym_out" [128, NITER] fp32 (per-partition sum-max partials)
    """
    import concourse.bass as bass
    import concourse.bacc as bacc
    import concourse.tile as tile
    from concourse import mybir
    from concourse.alu_op_type import AluOpType
    from concourse.dve_ops import (
        RECIPROCAL_APPROX_FAST,
        RECIP_APPROX_FAST_CONSTS,
    )

    f32 = mybir.dt.float32
    bf16 = mybir.dt.bfloat16
    f8 = mybir.dt.float8e4

    NV = YQc * Zc                 # voxels per x-slab
    assert NV % P == 0
    VT = NV // P                  # 128-voxel tiles per x-slab
    assert XS % (2 * CHUNK) == 0
    NITER = XS // (2 * CHUNK)
    CH = Cc // 2
    XV = CHUNK * VT               # voxel-groups per chunk (= 72)
    G = XV // U                   # gram groups per chunk (= 18)
    CSLAB = CHUNK * Cc * VT       # elements per chunk per partition

    nc = bacc.Bacc("TRN2", target_bir_lowering=False)
    lg_a = nc.dram_tensor("lg_a", [NITER, P, CSLAB], f8, kind="ExternalInput")
    lg_b = nc.dram_tensor("lg_b", [NITER, P, CSLAB], f8, kind="ExternalInput")
    a_out = nc.dram_tensor("a_out", [2, P, P], f32, kind="ExternalOutput")
    sym_out = nc.dram_tensor("sym_out", [1, 512], f32, kind="ExternalOutput")

    lg_dma_ring = []

    def load_chunk(pool, src, it, split):
        # one chunk half: [P, CHUNK, Cc, VT]; fully contiguous per partition.
        # split=True issues two half-DMAs so the first exp can start earlier
        # (pipeline ramp for iteration 0).
        t = pool.tile([P, CHUNK, Cc, VT], f8, tag="lg")
        HS = CSLAB // 2
        if split:
            for h in range(2):
                s = bass.AP(
                    tensor=src,
                    offset=it * P * CSLAB + h * HS,
                    ap=[[CSLAB, P], [1, HS]],
                )
                d = nc.sync.dma_start(
                    out=t[:, h * (CHUNK // 2) : (h + 1) * (CHUNK // 2)], in_=s
                )
                lg_dma_ring.append(d)
        else:
            s = bass.AP(
                tensor=src,
                offset=it * P * CSLAB,
                ap=[[CSLAB, P], [1, CSLAB]],
            )
            d = nc.sync.dma_start(out=t[:], in_=s)
            lg_dma_ring.append(d)
        return t

    with tile.TileContext(nc) as tc, ExitStack() as ctx:
        lg_pool = ctx.enter_context(tc.tile_pool(name="lg", bufs=4))
        e_pool = ctx.enter_context(tc.tile_pool(name="e", bufs=3))
        p_pool = ctx.enter_context(tc.tile_pool(name="p", bufs=3))
        st_pool = ctx.enter_context(tc.tile_pool(name="st", bufs=1))
        st34_pool = ctx.enter_context(tc.tile_pool(name="st34", bufs=2))
        sm_pool = ctx.enter_context(tc.tile_pool(name="sm", bufs=2))
        m_pool = ctx.enter_context(tc.tile_pool(name="m", bufs=2))
        one_pool = ctx.enter_context(tc.tile_pool(name="one", bufs=1))
        ps_pool = ctx.enter_context(tc.tile_pool(name="ps", bufs=1, space="PSUM"))

        a_psum = ps_pool.tile([P, P], f32)
        b_psum = ps_pool.tile([P, P], f32)
        sym_psum = ps_pool.tile([P, 512], f32)
        a_sb = one_pool.tile([P, P], f32)
        b_sb = one_pool.tile([P, P], f32)
        sym_sb = one_pool.tile([P, 512], f32)
        ones_t = one_pool.tile([P, 1], bf16)
        nc.vector.memset(ones_t[:], 1.0)

        n_mm = NITER * 2 * G
        state = {"mm": 0, "sym_mm": 0}
        F2 = 2 * XV               # tree batch size (both chunks, = 288)
        m_len = 2 * G * CH * U
        sym_offs = list(range(0, m_len, 512))
        n_sym_mm = len(sym_offs)

        def head(it):
            """DMA + exp + tree levels 1-2 (DVE) and 3-4 (GpSimd)."""
            split = it == 0
            lg_ta = load_chunk(lg_pool, lg_a, it, split)
            lg_tb = load_chunk(lg_pool, lg_b, it, split)

            # exp, channel-major out: e [P, Cc, j, (x, vt)]; iteration 0
            # splits each exp by x-half to shorten the pipeline ramp.
            e_t = e_pool.tile([P, Cc, 2, XV], bf16, tag="e")
            for j, lg_t in enumerate((lg_ta, lg_tb)):
                ev = e_t[:, :, j, :].rearrange("p c (x v) -> p c x v", v=VT)
                lv = lg_t[:].transpose([0, 2, 1, 3])
                if split:
                    hx = CHUNK // 2
                    for h in range(2):
                        nc.scalar.activation(
                            out=ev[:, :, h * hx : (h + 1) * hx, :],
                            in_=lv[:, :, h * hx : (h + 1) * hx, :],
                            func=mybir.ActivationFunctionType.Exp,
                        )
                else:
                    nc.scalar.activation(
                        out=ev, in_=lv,
                        func=mybir.ActivationFunctionType.Exp,
                    )

            # channel-sum tree: 32 -> 16 (DVE; j-split on iteration 0 so the
            # first add starts before the second chunk's exp lands)
            st1 = st_pool.tile([P, CH, 2, XV], bf16, tag="st1")
            if split:
                hu = XV // 2
                for j in range(2):
                    for h in range(2):
                        nc.vector.tensor_add(
                            st1[:, :, j, h * hu : (h + 1) * hu],
                            e_t[:, 0:CH, j, h * hu : (h + 1) * hu],
                            e_t[:, CH:Cc, j, h * hu : (h + 1) * hu],
                        )
            else:
                ef = e_t[:].rearrange("p c j u -> p (c j u)")
                n = CH * F2
                nc.vector.tensor_add(
                    st1[:].rearrange("p c j u -> p (c j u)"),
                    ef[:, 0:n],
                    ef[:, n : 2 * n],
                )
            # 16 -> 8 (DVE)
            st1f = st1[:].rearrange("p c j u -> p (c j u)")
            n = CH * F2 // 2
            st2 = st_pool.tile([P, n], bf16, tag="st2")
            nc.vector.tensor_add(st2[:], st1f[:, 0:n], st1f[:, n : 2 * n])
            # 8 -> 4 -> 2 (DVE)
            n //= 2
            st3 = st34_pool.tile([P, n], bf16, tag="st3")
            nc.vector.tensor_add(st3[:], st2[:, 0:n], st2[:, n : 2 * n])
            n //= 2
            st4 = st34_pool.tile([P, n], bf16, tag="st4")
            nc.vector.tensor_add(st4[:], st3[:, 0:n], st3[:, n : 2 * n])
            return e_t, st4

        def tail(it, e_t, st4):
            """Tree level 5 + reciprocal + normalize + symmetry + matmuls."""
            s_f = sm_pool.tile([P, F2], f32, tag="s")
            nc.vector.tensor_add(s_f[:], st4[:, 0:F2], st4[:, F2 : 2 * F2])

            # t = 1/s (fast NR approx, ~51 ULP), bf16 out
            t_b = sm_pool.tile([P, F2], bf16, tag="tb")
            _rc = RECIP_APPROX_FAST_CONSTS
            nc.vector._custom_dve(
                RECIPROCAL_APPROX_FAST,
                out=t_b[:],
                in0=s_f[:],
                s0=_rc["s0"],
                s1=_rc["s1"],
                imm2=_rc["imm2"],
            )

            # p = e * t; out layout [P, j, G, c, U].  On the last iteration
            # the mul is split by chunk so the j=0 gram matmuls start
            # draining the Tensor queue while the DVE finishes j=1 + max.
            p_t = p_pool.tile([P, 2, G, Cc, U], bf16, tag="p")
            p_w = p_t[:].rearrange("p j g c u -> p (j g) c u")
            e_r = e_t[:].rearrange("p c j (g u) -> p (j g) c u", u=U)
            t_r = (
                t_b[:]
                .rearrange("p (j g u) -> p (j g) u", j=2, u=U)
                .unsqueeze(2)
                .broadcast_to([P, 2 * G, Cc, U])
            )

            def emit_gram(j):
                dst = a_psum if j == 0 else b_psum
                for g in range(G):
                    pv = p_t[:, j, g].rearrange("p c u -> p (c u)")
                    nc.tensor.matmul(
                        dst[:],
                        pv,
                        pv,
                        start=(it == 0 and g == 0),
                        stop=(it == NITER - 1 and g == G - 1),
                    )
                    state["mm"] += 1

            last = it == NITER - 1
            if last:
                nc.vector.tensor_mul(
                    p_w[:, 0:G], e_r[:, 0:G], t_r[:, 0:G]
                )
                emit_gram(0)
                nc.vector.tensor_mul(
                    p_w[:, G : 2 * G], e_r[:, G : 2 * G], t_r[:, G : 2 * G]
                )
                emit_gram(1)
            else:
                nc.vector.tensor_mul(p_w, e_r, t_r)

            # symmetry: sum|pa - pb_sigma| = 2*sum max - const; the b chunk
            # is channel-half-swapped on host so this is one plain max
            m_t = m_pool.tile([P, 2, G, CH, U], bf16, tag="m")
            nc.vector.tensor_tensor(
                out=m_t[:].rearrange("p a g c u -> p (a g) c u"),
                in0=p_t[:, 0],
                in1=p_t[:, 1],
                op=AluOpType.max,
            )
            m_flat = m_t[:].rearrange("p a g c u -> p (a g c u)")
            for off in sym_offs:
                w = min(512, m_len - off)
                nc.tensor.matmul(
                    sym_psum[0:1, 0:w],
                    ones_t[:, 0:1],
                    m_flat[:, off : off + w],
                    start=(state["sym_mm"] == 0),
                    stop=(state["sym_mm"] == NITER * n_sym_mm - 1),
                )
                state["sym_mm"] += 1

            if not last:
                emit_gram(0)
                emit_gram(1)

        for it in range(NITER):
            e_t, st4 = head(it)
            tail(it, e_t, st4)

        assert state["mm"] == n_mm
        nc.vector.tensor_copy(out=a_sb[:], in_=a_psum[:])
        nc.sync.dma_start(out=a_out[0], in_=a_sb[:])
        nc.vector.tensor_copy(out=b_sb[:], in_=b_psum[:])
        nc.sync.dma_start(out=a_out[1], in_=b_sb[:])
        nc.vector.tensor_copy(out=sym_sb[0:1, :], in_=sym_psum[0:1, :])
        nc.sync.dma_start(out=sym_out[:], in_=sym_sb[0:1, :])

    # The HWDGE pseudo-DMA has a single sync-wait slot, but a recycled load
    # buffer carries both a WAR wait and a WAW wait.  All SP-issued HWDGE
    # DMAs share one physical FIFO ring, so same-ring WAW ordering is
    # guaranteed by hardware; drop the redundant DMAHW wait.
    for d in lg_dma_ring:
        si = d.ins.sync_info
        if si is None or si.on_wait is None:
            continue
        ws = list(si.on_wait)
        if len(ws) > 1:
            keep = [w for w in ws if not (w.ant_name or "").startswith("DMAHW")]
            if keep and len(keep) < len(ws):
                si.on_wait = keep

    nc.compile()
    return nc


def _finish_loss(A_b, vol_b, sym_total, age, w_young, w_old,
                 vol_means_young, vol_means_old, vol_stds_young, vol_stds_old,
                 prior_adj):
    """Host-side tiny final math (numpy, float64 internally)."""
    alpha = np.clip(age.astype(np.float64) / AGE_MAX, 0.0, 1.0)  # (B,1)

    eye = np.eye(C)
    A = A_b * (1.0 - eye)[None]                                   # zero diag
    W = (1.0 - alpha)[:, :, None] * w_young[None] + alpha[:, :, None] * w_old[None]
    Aw = (A * W).mean(axis=0)
    Aw = Aw / np.clip(Aw.sum(axis=1, keepdims=True), EPS_ROW, None)
    prior = prior_adj * (1.0 - eye)
    prior = prior / np.clip(prior.sum(axis=1, keepdims=True), EPS_ROW, None)
    loss_adj = np.mean(np.abs(Aw - prior))

    means = (1.0 - alpha) * vol_means_young[None] + alpha * vol_means_old[None]
    stds = (1.0 - alpha) * vol_stds_young[None] + alpha * vol_stds_old[None]
    r = (vol_b - means) / (stds + EPS_STD)
    ar = np.abs(r)
    loss_vol = np.mean(np.where(ar < 1.0, 0.5 * r * r, ar - 0.5))

    loss_sym = sym_total / float(B * C * X * Y * Z)

    total = (LAMBDA_WEIGHTED_ADJ * loss_adj
             + LAMBDA_VOLUME * loss_vol
             + LAMBDA_SYM * loss_sym)
    return np.float32(total)


def _shard_for_core(logits, b, q, Cc=C, XS=X, YQc=YQ, Zc=Z):
    """Slice one core's shard into (lg_a, lg_b): ascending / descending
    chunk tensors [NITER, 128, CHUNK*C*VT] bf16 with voxel v = y*Zc + z
    mapped to (vt, part) = (v // 128, v % 128)."""
    NV = YQc * Zc
    VT = NV // P
    NITER = XS // (2 * CHUNK)
    sh = logits[b, :, :, q * YQc : (q + 1) * YQc, :]      # [C, XS, YQ, Z]
    sh = sh.reshape(Cc, XS, VT, P)                        # v -> (vt, part)
    sh = sh.transpose(1, 3, 0, 2)                         # [XS, part, C, VT]
    import ml_dtypes
    sh = np.asarray(sh, dtype=np.float32).astype(ml_dtypes.float8_e4m3)
    asc = sh[: XS // 2].reshape(NITER, CHUNK, P, Cc, VT)
    # descending shard: swap channel halves (the LR pair permutation) so the
    # on-device symmetry max needs no swizzled access pattern
    perm = np.concatenate([np.arange(Cc // 2, Cc), np.arange(0, Cc // 2)])
    dsc = sh[XS // 2 :][::-1][:, :, perm]
    dsc = dsc.reshape(NITER, CHUNK, P, Cc, VT)
    # [NITER, P, CHUNK, C, VT] flattened per partition
    lg_a = np.ascontiguousarray(asc.transpose(0, 2, 1, 3, 4)).reshape(
        NITER, P, CHUNK * Cc * VT
    )
    lg_b = np.ascontiguousarray(dsc.transpose(0, 2, 1, 3, 4)).reshape(
        NITER, P, CHUNK * Cc * VT
    )
    return lg_a, lg_b


_CACHE = {}


def kernel(logits, age, w_young, w_old, vol_means_young, vol_means_old,
           vol_stds_young, vol_stds_old, prior_adj, perm):
    from concourse.bass_utils import run_bass_kernel_spmd

    logits = np.asarray(logits, dtype=np.float32)

    if "nc" not in _CACHE:
        _CACHE["nc"] = build_nc()
    nc = _CACHE["nc"]

    in_maps = []
    for core in range(N_CORES):
        b = core // 4
        q = core % 4
        la, lb = _shard_for_core(logits, b, q)
        in_maps.append({"lg_a": la, "lg_b": lb})

    res = run_bass_kernel_spmd(nc, in_maps, core_ids=list(range(N_CORES)))
    _CACHE["last_results"] = res

    NVOX_CORE = X * YQ * Z
    A_b = np.zeros((B, C, C), dtype=np.float64)
    sym_total = 0.0
    for core in range(N_CORES):
        b = core // 4
        a_full = res.results[core]["a_out"].astype(np.float64)
        # a_full[j, 4*c1+u1, 4*c2+u2]: diagonal u1==u2 blocks are the gram;
        # the j=1 (descending) gram is channel-half-swapped -> unpermute
        perm = np.concatenate([np.arange(C // 2, C), np.arange(0, C // 2)])
        Aa = np.einsum("cudu->cd", a_full[0].reshape(C, U, C, U))
        Ab = np.einsum("cudu->cd", a_full[1].reshape(C, U, C, U))
        A_b[b] += Aa + Ab[np.ix_(perm, perm)]
        sum_max = float(res.results[core]["sym_out"].astype(np.float64).sum())
        sym_core = 2.0 * sum_max - NVOX_CORE
        sym_total += 2.0 * sym_core
    vol_b = A_b.sum(axis=2)  # softmax rows sum to 1 -> row sums give volumes

    return _finish_loss(
        A_b, vol_b, sym_total,
        np.asarray(age), np.asarray(w_young), np.asarray(w_old),
        np.asarray(vol_means_young), np.asarray(vol_means_old),
        np.asarray(vol_stds_young), np.asarray(vol_stds_old),
        np.asarray(prior_adj),
    )


# revision 26
# speedup vs baseline: 1.0391x; 1.0362x over previous
"""Trainium2 Bass kernel for nn_AgeConditionedGraphPriorLoss.

Strategy (final)
----------------
logits (2, 32, 96, 96, 96) fp32 is the only large tensor (~216 MiB); the
problem is memory-bound.  Shard over (batch B=2) x (four Y-slabs of 24)
across 8 NeuronCores; each core keeps the full X range so the flip/swap
symmetry term is shard-local.

Host prep: shards are pre-transposed to [NITER, 128, CHUNK*C*VT], cast to
fp8 e4m3 (softmax normalizes the same quantized values, so row sums are
still exactly 1), and the descending-x half has its channel halves
pre-swapped (the LR pair permutation), so every device access pattern is
contiguous.

Per core, NITER=6 iterations process an (x ascending, x descending)
chunk pair of CHUNK=8 slabs each:
  * ACT:  e = exp(logit) -> bf16, channel-major [P, C, j, (x,vt)]
  * DVE:  s = sum_c e as a 5-level binary tree of contiguous halving
          bf16 adds (2x packing); t = 1/s via the custom
          reciprocal_approx_fast op, emitted straight to bf16;
          p = e * t in one mul with t broadcast over channels
  * DVE:  symmetry via sum|a-b| = 2*sum max(a,b) - sum a - sum b; with
          softmax rows summing to 1, sum a + sum b is the exact voxel
          count, so one bf16 max per iteration is all the element work
  * PE:   sum max reduced by ones-vector matmuls accumulating into a
          [1,512] PSUM row; gram matmuls packed 4 vtiles wide
          ([128,128]^T[128,128], 432 total) into two PSUM banks (the
          descending chunk's gram is channel-swapped; host unpermutes)
Volumes are gram row sums (softmax rows sum to 1).  Iteration 0 is
split into half-DMAs / half-exps / quarter-L1s to shorten the pipeline
ramp, and the last iteration reorders mul/gram so the Tensor queue
drains early.  The tiny O(C^2) final loss math runs on host in numpy.
"""

import os
import sys

import numpy as np
from contextlib import ExitStack

# kernel.py is graded from a bare directory: make the concourse/bass stack
# importable regardless of cwd
for _p in ("/opt/trn_rl_repo", "/root/.axon_site/_ro/trn_rl_repo"):
    if os.path.isdir(_p) and _p not in sys.path:
        sys.path.append(_p)

# ---- problem constants (hardcoded per harness contract) ----
B = 2
C = 32
X = 96
Y = 96
Z = 96
N_CORES = 8
YQ = Y // 4          # y-slab per core
P = 128              # SBUF partitions

LAMBDA_VOLUME = 0.2
LAMBDA_WEIGHTED_ADJ = 0.15
LAMBDA_SYM = 0.05
AGE_MAX = 100.0
EPS_ROW = 1e-8
EPS_STD = 1e-6

CHUNK = 8            # x-slabs per chunk half
U = 4                # vtiles packed per gram matmul


def build_nc(Cc=C, XS=X, YQc=YQ, Zc=Z):
    """Build the per-core Bass program (SPMD: same program on all cores).

    Inputs : "lg_a" [NITER, 128, CHUNK*Cc*VT] bf16  (ascending x chunks)
             "lg_b" [NITER, 128, CHUNK*Cc*VT] bf16  (descending x chunks)
    Outputs: "a_out"   [128, 128] fp32  (packed gram blocks, diag extract)
             "sym_out" [128, NITER] fp32 (per-partition sum-max partials)
    """
    import concourse.bass as bass
    import concourse.bacc as bacc
    import concourse.tile as tile
    from concourse import mybir
    from concourse.alu_op_type import AluOpType
    from concourse.dve_ops import (
        RECIP_APPROX_FAST_CONSTS,
        _SUB_OPCODE_FOR_NAME,
        CUSTOM_DVE_SPECS,
        DveOp,
        OPS,
    )
    from concourse import dve_spec as DS
    from concourse.dve_uop import DveVer

    # RECIP_SUM_ANT: out = 1/(Src0+Src1) via the BITWISE_NOT exponent-flip
    # seed + ONE Newton-Raphson pass + a bias-centering final scale (the
    # 2-NR chain plus the add exceeds the 8-slice budget).  ~0.3% max err,
    # bias-centered; t feeds a bf16 multiply so this is ample.
    def _make_recip_sum():
        name = "RECIP_SUM_ANT"
        for op in OPS:
            if op.name == name:
                return op
        _x = DS.Src0 + DS.Src1
        _nx = DS.Bin(DS.AluOp.BITWISE_NOT, _x, _x)
        _y0 = _nx * DS.C0

        def _ref(in0, in1, c0, c1, c2):
            x = (in0.astype(np.float32) + in1.astype(np.float32))
            nx = (~x.view(np.int32)).view(np.float32)
            y0 = nx * c0
            return y0 * (c1 - x * y0) * c2

        spec = DS.Spec(body=_y0 * (DS.C1 - _x * _y0) * DS.C2, reference=_ref)
        row = max(_SUB_OPCODE_FOR_NAME.values()) + 1
        _SUB_OPCODE_FOR_NAME[name] = row
        CUSTOM_DVE_SPECS[name] = spec
        op = DveOp(name, spec, subdim=False, uops_sha={})
        # discover the uops sha (pinned-sha check raises with the actual)
        import re as _re
        shas = {}
        for ver in ("v3", "v4"):
            try:
                op.compile(ver)
            except ValueError as e:
                m = _re.search(r"\(v\d: (\w+) ", str(e))
                if m:
                    shas[ver] = m.group(1)
            except Exception:
                pass
        op = DveOp(name, spec, subdim=False, uops_sha=shas)
        OPS.append(op)
        return op

    RECIP_SUM = _make_recip_sum()

    f32 = mybir.dt.float32
    bf16 = mybir.dt.bfloat16
    f8 = mybir.dt.float8e4

    NV = YQc * Zc                 # voxels per x-slab
    assert NV % P == 0
    VT = NV // P                  # 128-voxel tiles per x-slab
    assert XS % (2 * CHUNK) == 0
    NITER = XS // (2 * CHUNK)
    CH = Cc // 2
    XV = CHUNK * VT               # voxel-groups per chunk (= 72)
    G = XV // U                   # gram groups per chunk (= 18)
    CSLAB = CHUNK * Cc * VT       # elements per chunk per partition

    nc = bacc.Bacc("TRN2", target_bir_lowering=False)
    lg_a = nc.dram_tensor("lg_a", [NITER, P, CSLAB], f8, kind="ExternalInput")
    lg_b = nc.dram_tensor("lg_b", [NITER, P, CSLAB], f8, kind="ExternalInput")
    a_out = nc.dram_tensor("a_out", [2, P, P], f32, kind="ExternalOutput")
    sym_out = nc.dram_tensor("sym_out", [1, 512], f32, kind="ExternalOutput")

    lg_dma_ring = []

    def load_chunk(pool, src, it, split):
        # one chunk half: [P, CHUNK, Cc, VT]; fully contiguous per partition.
        # split=True issues two half-DMAs so the first exp can start earlier
        # (pipeline ramp for iteration 0).
        t = pool.tile([P, CHUNK, Cc, VT], f8, tag="lg")
        HS = CSLAB // 2
        if split:
            for h in range(2):
                s = bass.AP(
                    tensor=src,
                    offset=it * P * CSLAB + h * HS,
                    ap=[[CSLAB, P], [1, HS]],
                )
                d = nc.sync.dma_start(
                    out=t[:, h * (CHUNK // 2) : (h + 1) * (CHUNK // 2)], in_=s
                )
                lg_dma_ring.append(d)
        else:
            s = bass.AP(
                tensor=src,
                offset=it * P * CSLAB,
                ap=[[CSLAB, P], [1, CSLAB]],
            )
            d = nc.sync.dma_start(out=t[:], in_=s)
            lg_dma_ring.append(d)
        return t

    with tile.TileContext(nc) as tc, ExitStack() as ctx:
        lg_pool = ctx.enter_context(tc.tile_pool(name="lg", bufs=4))
        e_pool = ctx.enter_context(tc.tile_pool(name="e", bufs=3))
        p_pool = ctx.enter_context(tc.tile_pool(name="p", bufs=3))
        st_pool = ctx.enter_context(tc.tile_pool(name="st", bufs=1))
        st34_pool = ctx.enter_context(tc.tile_pool(name="st34", bufs=2))
        sm_pool = ctx.enter_context(tc.tile_pool(name="sm", bufs=2))
        m_pool = ctx.enter_context(tc.tile_pool(name="m", bufs=2))
        one_pool = ctx.enter_context(tc.tile_pool(name="one", bufs=1))
        ps_pool = ctx.enter_context(tc.tile_pool(name="ps", bufs=1, space="PSUM"))

        a_psum = ps_pool.tile([P, P], f32)
        b_psum = ps_pool.tile([P, P], f32)
        sym_psum = ps_pool.tile([P, 512], f32)
        a_sb = one_pool.tile([P, P], f32)
        b_sb = one_pool.tile([P, P], f32)
        sym_sb = one_pool.tile([P, 512], f32)
        ones_t = one_pool.tile([P, 1], bf16)
        nc.vector.memset(ones_t[:], 1.0)

        n_mm = NITER * 2 * G
        state = {"mm": 0, "sym_mm": 0}
        F2 = 2 * XV               # tree batch size (both chunks, = 288)
        m_len = 2 * G * CH * U
        sym_offs = list(range(0, m_len, 512))
        n_sym_mm = len(sym_offs)

        def head(it):
            """DMA + exp + tree levels 1-2 (DVE) and 3-4 (GpSimd)."""
            split = it == 0
            lg_ta = load_chunk(lg_pool, lg_a, it, split)
            lg_tb = load_chunk(lg_pool, lg_b, it, split)

            # exp, channel-major out: e [P, Cc, j, (x, vt)]; iteration 0
            # splits each exp by x-half to shorten the pipeline ramp.
            e_t = e_pool.tile([P, Cc, 2, XV], bf16, tag="e")
            for j, lg_t in enumerate((lg_ta, lg_tb)):
                ev = e_t[:, :, j, :].rearrange("p c (x v) -> p c x v", v=VT)
                lv = lg_t[:].transpose([0, 2, 1, 3])
                if split:
                    hx = CHUNK // 2
                    for h in range(2):
                        nc.scalar.activation(
                            out=ev[:, :, h * hx : (h + 1) * hx, :],
                            in_=lv[:, :, h * hx : (h + 1) * hx, :],
                            func=mybir.ActivationFunctionType.Exp,
                        )
                else:
                    nc.scalar.activation(
                        out=ev, in_=lv,
                        func=mybir.ActivationFunctionType.Exp,
                    )

            # channel-sum tree: 32 -> 16 (DVE; j-split on iteration 0 so the
            # first add starts before the second chunk's exp lands)
            st1 = st_pool.tile([P, CH, 2, XV], bf16, tag="st1")
            if split:
                hu = XV // 2
                for j in range(2):
                    for h in range(2):
                        nc.vector.tensor_add(
                            st1[:, :, j, h * hu : (h + 1) * hu],
                            e_t[:, 0:CH, j, h * hu : (h + 1) * hu],
                            e_t[:, CH:Cc, j, h * hu : (h + 1) * hu],
                        )
            else:
                ef = e_t[:].rearrange("p c j u -> p (c j u)")
                n = CH * F2
                nc.vector.tensor_add(
                    st1[:].rearrange("p c j u -> p (c j u)"),
                    ef[:, 0:n],
                    ef[:, n : 2 * n],
                )
            # 16 -> 8 (DVE)
            st1f = st1[:].rearrange("p c j u -> p (c j u)")
            n = CH * F2 // 2
            st2 = st_pool.tile([P, n], bf16, tag="st2")
            nc.vector.tensor_add(st2[:], st1f[:, 0:n], st1f[:, n : 2 * n])
            # 8 -> 4 -> 2 (DVE)
            n //= 2
            st3 = st34_pool.tile([P, n], bf16, tag="st3")
            nc.vector.tensor_add(st3[:], st2[:, 0:n], st2[:, n : 2 * n])
            n //= 2
            st4 = st34_pool.tile([P, n], bf16, tag="st4")
            nc.vector.tensor_add(st4[:], st3[:, 0:n], st3[:, n : 2 * n])
            return e_t, st4

        def tail(it, e_t, st4):
            """Tree level 5 + reciprocal + normalize + symmetry + matmuls."""
            # t = 1/(st4_lo + st4_hi) fused in one custom DVE op
            t_b = sm_pool.tile([P, F2], bf16, tag="tb")
            _rc = RECIP_APPROX_FAST_CONSTS
            nc.vector._custom_dve(
                RECIP_SUM,
                out=t_b[:],
                in0=st4[:, 0:F2],
                in1=st4[:, F2 : 2 * F2],
                s0=_rc["s0"],
                s1=_rc["s1"],
                imm2=1.0012,
            )

            # p = e * t; out layout [P, j, G, c, U].  On the last iteration
            # the mul is split by chunk so the j=0 gram matmuls start
            # draining the Tensor queue while the DVE finishes j=1 + max.
            p_t = p_pool.tile([P, 2, G, Cc, U], bf16, tag="p")
            p_w = p_t[:].rearrange("p j g c u -> p (j g) c u")
            e_r = e_t[:].rearrange("p c j (g u) -> p (j g) c u", u=U)
            t_r = (
                t_b[:]
                .rearrange("p (j g u) -> p (j g) u", j=2, u=U)
                .unsqueeze(2)
                .broadcast_to([P, 2 * G, Cc, U])
            )

            def emit_gram(j):
                dst = a_psum if j == 0 else b_psum
                for g in range(G):
                    pv = p_t[:, j, g].rearrange("p c u -> p (c u)")
                    nc.tensor.matmul(
                        dst[:],
                        pv,
                        pv,
                        start=(it == 0 and g == 0),
                        stop=(it == NITER - 1 and g == G - 1),
                    )
                    state["mm"] += 1

            last = it == NITER - 1
            if last:
                nc.vector.tensor_mul(
                    p_w[:, 0:G], e_r[:, 0:G], t_r[:, 0:G]
                )
                emit_gram(0)
                nc.vector.tensor_mul(
                    p_w[:, G : 2 * G], e_r[:, G : 2 * G], t_r[:, G : 2 * G]
                )
                emit_gram(1)
            else:
                nc.vector.tensor_mul(p_w, e_r, t_r)

            # symmetry: sum|pa - pb_sigma| = 2*sum max - const; the b chunk
            # is channel-half-swapped on host so this is one plain max
            m_t = m_pool.tile([P, 2, G, CH, U], bf16, tag="m")
            nc.vector.tensor_tensor(
                out=m_t[:].rearrange("p a g c u -> p (a g) c u"),
                in0=p_t[:, 0],
                in1=p_t[:, 1],
                op=AluOpType.max,
            )
            m_flat = m_t[:].rearrange("p a g c u -> p (a g c u)")
            for off in sym_offs:
                w = min(512, m_len - off)
                nc.tensor.matmul(
                    sym_psum[0:1, 0:w],
                    ones_t[:, 0:1],
                    m_flat[:, off : off + w],
                    start=(state["sym_mm"] == 0),
                    stop=(state["sym_mm"] == NITER * n_sym_mm - 1),
                )
                state["sym_mm"] += 1

            if not last:
                emit_gram(0)
                emit_gram(1)

        for it in range(NITER):
            e_t, st4 = head(it)
            tail(it, e_t, st4)

        assert state["mm"] == n_mm
        nc.vector.tensor_copy(out=a_sb[:], in_=a_psum[:])
        nc.sync.dma_start(out=a_out[0], in_=a_sb[:])
        nc.vector.tensor_copy(out=b_sb[:], in_=b_psum[:])
        nc.sync.dma_start(out=a_out[1], in_=b_sb[:])
        nc.vector.tensor_copy(out=sym_sb[0:1, :], in_=sym_psum[0:1, :])
        nc.sync.dma_start(out=sym_out[:], in_=sym_sb[0:1, :])

    # The HWDGE pseudo-DMA has a single sync-wait slot, but a recycled load
    # buffer carries both a WAR wait and a WAW wait.  All SP-issued HWDGE
    # DMAs share one physical FIFO ring, so same-ring WAW ordering is
    # guaranteed by hardware; drop the redundant DMAHW wait.
    for d in lg_dma_ring:
        si = d.ins.sync_info
        if si is None or si.on_wait is None:
            continue
        ws = list(si.on_wait)
        if len(ws) > 1:
            keep = [w for w in ws if not (w.ant_name or "").startswith("DMAHW")]
            if keep and len(keep) < len(ws):
                si.on_wait = keep

    nc.compile()
    return nc


def _finish_loss(A_b, vol_b, sym_total, age, w_young, w_old,
                 vol_means_young, vol_means_old, vol_stds_young, vol_stds_old,
                 prior_adj):
    """Host-side tiny final math (numpy, float64 internally)."""
    alpha = np.clip(age.astype(np.float64) / AGE_MAX, 0.0, 1.0)  # (B,1)

    eye = np.eye(C)
    A = A_b * (1.0 - eye)[None]                                   # zero diag
    W = (1.0 - alpha)[:, :, None] * w_young[None] + alpha[:, :, None] * w_old[None]
    Aw = (A * W).mean(axis=0)
    Aw = Aw / np.clip(Aw.sum(axis=1, keepdims=True), EPS_ROW, None)
    prior = prior_adj * (1.0 - eye)
    prior = prior / np.clip(prior.sum(axis=1, keepdims=True), EPS_ROW, None)
    loss_adj = np.mean(np.abs(Aw - prior))

    means = (1.0 - alpha) * vol_means_young[None] + alpha * vol_means_old[None]
    stds = (1.0 - alpha) * vol_stds_young[None] + alpha * vol_stds_old[None]
    r = (vol_b - means) / (stds + EPS_STD)
    ar = np.abs(r)
    loss_vol = np.mean(np.where(ar < 1.0, 0.5 * r * r, ar - 0.5))

    loss_sym = sym_total / float(B * C * X * Y * Z)

    total = (LAMBDA_WEIGHTED_ADJ * loss_adj
             + LAMBDA_VOLUME * loss_vol
             + LAMBDA_SYM * loss_sym)
    return np.float32(total)


def _shard_for_core(logits, b, q, Cc=C, XS=X, YQc=YQ, Zc=Z):
    """Slice one core's shard into (lg_a, lg_b): ascending / descending
    chunk tensors [NITER, 128, CHUNK*C*VT] bf16 with voxel v = y*Zc + z
    mapped to (vt, part) = (v // 128, v % 128)."""
    NV = YQc * Zc
    VT = NV // P
    NITER = XS // (2 * CHUNK)
    sh = logits[b, :, :, q * YQc : (q + 1) * YQc, :]      # [C, XS, YQ, Z]
    sh = sh.reshape(Cc, XS, VT, P)                        # v -> (vt, part)
    sh = sh.transpose(1, 3, 0, 2)                         # [XS, part, C, VT]
    import ml_dtypes
    sh = np.asarray(sh, dtype=np.float32).astype(ml_dtypes.float8_e4m3)
    asc = sh[: XS // 2].reshape(NITER, CHUNK, P, Cc, VT)
    # descending shard: swap channel halves (the LR pair permutation) so the
    # on-device symmetry max needs no swizzled access pattern
    perm = np.concatenate([np.arange(Cc // 2, Cc), np.arange(0, Cc // 2)])
    dsc = sh[XS // 2 :][::-1][:, :, perm]
    dsc = dsc.reshape(NITER, CHUNK, P, Cc, VT)
    # [NITER, P, CHUNK, C, VT] flattened per partition
    lg_a = np.ascontiguousarray(asc.transpose(0, 2, 1, 3, 4)).reshape(
        NITER, P, CHUNK * Cc * VT
    )
    lg_b = np.ascontiguousarray(dsc.transpose(0, 2, 1, 3, 4)).reshape(
        NITER, P, CHUNK * Cc * VT
    )
    return lg_a, lg_b


_CACHE = {}


def kernel(logits, age, w_young, w_old, vol_means_young, vol_means_old,
           vol_stds_young, vol_stds_old, prior_adj, perm):
    from concourse.bass_utils import run_bass_kernel_spmd

    logits = np.asarray(logits, dtype=np.float32)

    if "nc" not in _CACHE:
        _CACHE["nc"] = build_nc()
    nc = _CACHE["nc"]

    in_maps = []
    for core in range(N_CORES):
        b = core // 4
        q = core % 4
        la, lb = _shard_for_core(logits, b, q)
        in_maps.append({"lg_a": la, "lg_b": lb})

    res = run_bass_kernel_spmd(nc, in_maps, core_ids=list(range(N_CORES)))
    _CACHE["last_results"] = res

    NVOX_CORE = X * YQ * Z
    A_b = np.zeros((B, C, C), dtype=np.float64)
    sym_total = 0.0
    for core in range(N_CORES):
        b = core // 4
        a_full = res.results[core]["a_out"].astype(np.float64)
        # a_full[j, 4*c1+u1, 4*c2+u2]: diagonal u1==u2 blocks are the gram;
        # the j=1 (descending) gram is channel-half-swapped -> unpermute
        perm = np.concatenate([np.arange(C // 2, C), np.arange(0, C // 2)])
        Aa = np.einsum("cudu->cd", a_full[0].reshape(C, U, C, U))
        Ab = np.einsum("cudu->cd", a_full[1].reshape(C, U, C, U))
        A_b[b] += Aa + Ab[np.ix_(perm, perm)]
        sum_max = float(res.results[core]["sym_out"].astype(np.float64).sum())
        sym_core = 2.0 * sum_max - NVOX_CORE
        sym_total += 2.0 * sym_core
    vol_b = A_b.sum(axis=2)  # softmax rows sum to 1 -> row sums give volumes

    return _finish_loss(
        A_b, vol_b, sym_total,
        np.asarray(age), np.asarray(w_young), np.asarray(w_old),
        np.asarray(vol_means_young), np.asarray(vol_means_old),
        np.asarray(vol_stds_young), np.asarray(vol_stds_old),
        np.asarray(prior_adj),
    )
